# revision 12
# baseline (speedup 1.0000x reference)
"""Trainium2 Bass kernel for the ConcreteLayer training forward pass.

Computes out = x @ softmax((weight - ln(-ln((1-tiny)*uniform + tiny))) / T, axis=1)

Strategy (8 NeuronCores, 4x2 grid):
  - 4 batch groups x 2 out-column halves; core = 2*p + q.
  - Chunk-pipelined softmax: Ln+Ln on scalar, (w - m) subtract on gpsimd,
    wide Exp on scalar, per-chunk 3D tensor_reduce row sums on vector.
  - Row sums exchanged between column-half siblings in fine-grained
    AllGather groups; each group's normalize (vector) + matmuls (tensor)
    are emitted with a ~1-chunk delay so the GEMM runs concurrently with
    softmax production instead of after it.
  - GEMM: bf16 lhsT (host-transposed x slice) x normalized bf16 e,
    f32 PSUM, 8 banks (one per 128-row output tile).
  - Output stored as bf16 (upcast on host) to trim HBM store traffic.
"""

import sys

import numpy as np

for _p in ("/opt/trn_rl_repo",):
    if _p not in sys.path:
        sys.path.insert(0, _p)

B, IN, OUT = 4096, 4096, 1024
GB, GO = 4, 2  # batch groups x out-half groups
BS = B // GB  # 1024 batch rows per core
OH = OUT // GO  # 512 out cols per core
P = 128
KT = IN // P  # 32 contraction tiles
KG = 4  # ktiles per softmax chunk
VSCALE = 16384.0  # 2^14 host pre-scale on v' = 1-u (fp16 encoding)
NCH = KT // KG  # 8 chunks
MBT = BS // P  # 8 output row tiles per core
NCORES = 8
GROUPS = [2, 6, 8, 8, 4, 4]  # ktiles per row-sum exchange group
# group -> exp-chunk index after which finish(g) (norm+GEMM) is emitted
FIN_AT = {0: 1, 1: 2, 2: 4, 3: 6, 4: 7}
TINY = float(np.finfo(np.float32).tiny)

_PROGRAM = None
LAST_RESULT = None


def _pin_act_tables():
    """Steer the act-table-load pass to one set (has both Ln and Exp) so the
    compiler emits one ACT_TABLE_LOAD instead of reloading per tile."""
    import concourse.mybir as mybir
    from concourse import bacc, hw_specs

    orig = hw_specs.get_activation_tables.__wrapped__
    target = "natural_log_exp_and_others"
    strip = {
        mybir.ActivationFunctionType.Ln,
        mybir.ActivationFunctionType.Exp,
    }

    def pinned(arch):
        tables = orig(arch)
        if target not in tables:
            return tables
        return {
            name: (set(fns) if name == target else {f for f in fns if f not in strip})
            for name, fns in tables.items()
        }

    bacc.get_activation_tables = pinned


def _build_program():
    import concourse.bass as bass
    import concourse.mybir as mybir
    import concourse.tile as tile
    from concourse import bacc
    from contextlib import ExitStack

    _pin_act_tables()

    f32 = mybir.dt.float32
    f16 = mybir.dt.float16
    bf16 = mybir.dt.bfloat16
    Ln = mybir.ActivationFunctionType.Ln
    Exp = mybir.ActivationFunctionType.Exp
    Alu = mybir.AluOpType

    nc = bacc.Bacc(
        "TRN2", target_bir_lowering=False, debug=False, num_devices=NCORES
    )

    xt_d = nc.dram_tensor("xt", [IN, BS], bf16, kind="ExternalInput")
    wh_d = nc.dram_tensor("wh", [IN, OH], bf16, kind="ExternalInput")
    uh_d = nc.dram_tensor("uh", [IN, OH], f16, kind="ExternalInput")
    t_d = nc.dram_tensor("tt", [1], f32, kind="ExternalInput")
    out_d = nc.dram_tensor("out", [BS, OH], bf16, kind="ExternalOutput")

    replica_groups = [[0, 1], [2, 3], [4, 5], [6, 7]]
    NG = len(GROUPS)
    gbounds = []
    s = 0
    for gsz in GROUPS:
        gbounds.append((s, s + gsz))
        s += gsz
    assert s == KT

    with tile.TileContext(nc) as tc, ExitStack() as ctx:
        dram = ctx.enter_context(tc.tile_pool(name="dram", bufs=1, space="DRAM"))
        singles = ctx.enter_context(tc.tile_pool(name="singles", bufs=1))
        chunks = ctx.enter_context(tc.tile_pool(name="chunks", bufs=4))
        outp = ctx.enter_context(tc.tile_pool(name="outp", bufs=4))
        psum = ctx.enter_context(tc.tile_pool(name="psum", bufs=1, space="PSUM"))

        # 1/T broadcast to all partitions.
        t_sb = singles.tile([P, 1], f32)
        t_ap = t_d.ap()
        nc.sync.dma_start(
            out=t_sb, in_=bass.AP(tensor=t_ap.tensor, offset=0, ap=[[0, P], [1, 1]])
        )
        invt = singles.tile([P, 1], f32)
        nc.vector.reciprocal(invt, t_sb)

        zero_t = singles.tile([P, 1], f32)
        nc.vector.memset(zero_t, 0.0)
        one_t = singles.tile([P, 1], f32)
        nc.vector.memset(one_t, 1.0)

        # Resident tensors.
        xt_all = singles.tile([P, KT, BS], bf16)
        e_all = singles.tile([P, KT, OH], bf16)
        sums = singles.tile([P, KT, 1], f32)
        invr = singles.tile([P, KT], f32)

        cc_in = [
            dram.tile([P, gsz], f32, name=f"cc_in{g}", tag=f"cc_in{g}")
            for g, gsz in enumerate(GROUPS)
        ]
        cc_out = [
            dram.tile([2, P, gsz], f32, name=f"cc_out{g}", tag=f"cc_out{g}")
            for g, gsz in enumerate(GROUPS)
        ]

        ps_tiles = [
            psum.tile([P, OH], f32, tag=f"ps{mb}", name=f"ps{mb}")
            for mb in range(MBT)
        ]

        def chunk_front(kb):
            """u/w DMA + the two Ln passes for chunk kb."""
            base = kb * KG * P
            u_t = chunks.tile([P, KG, OH], f16, tag="u", name="u_t")
            l_t = chunks.tile([P, KG, OH], f32, tag="l", name="l_t")
            w_t = chunks.tile([P, KG, OH], bf16, tag="w", name="w_t")
            u_src = uh_d[base : base + KG * P, :].rearrange("(g p) c -> p g c", p=P)
            w_src = wh_d[base : base + KG * P, :].rearrange("(g p) c -> p g c", p=P)
            nc.sync.dma_start(out=u_t, in_=u_src)
            nc.scalar.dma_start(out=w_t, in_=w_src)
            # l = ln(1 - (1-tiny)*2^-14 * v')  ==  ln((1-tiny)*u + tiny) < 0
            nc.scalar.activation(
                l_t, u_t, Ln, bias=one_t[:], scale=-(1.0 - TINY) / VSCALE
            )
            # m = ln(-l) = -gumbel
            nc.scalar.activation(l_t, l_t, Ln, bias=zero_t[:], scale=-1.0)
            return l_t, w_t

        def chunk_mid(kb, u_t, w_t):
            """gpsimd: d = w - m for chunk kb (f32, in place in u_t)."""
            nc.gpsimd.tensor_sub(u_t, w_t, u_t)

        def chunk_exp(kb, u_t):
            """Wide Exp into e_all + per-chunk row sums for chunk kb."""
            nc.scalar.activation(
                e_all[:, kb * KG : (kb + 1) * KG, :],
                u_t,
                Exp,
                bias=zero_t[:],
                scale=invt[:],
            )
            nc.vector.tensor_reduce(
                sums[:, kb * KG : (kb + 1) * KG, :],
                e_all[:, kb * KG : (kb + 1) * KG, :],
                mybir.AxisListType.X,
                Alu.add,
            )

        def xt_load(xb):
            """One 4-ktile chunk of the lhsT, on the gpsimd queue."""
            base = xb * KG * P
            src = xt_d[base : base + KG * P, :].rearrange("(g p) b -> p g b", p=P)
            nc.gpsimd.dma_start(out=xt_all[:, xb * KG : (xb + 1) * KG, :], in_=src)

        def exchange(g):
            gs, ge = gbounds[g]
            nc.sync.dma_start(out=cc_in[g], in_=sums[:, gs:ge, 0])
            nc.gpsimd.collective_compute(
                "AllGather",
                Alu.bypass,
                replica_groups=replica_groups,
                ins=[cc_in[g].opt()],
                outs=[cc_out[g].opt()],
            )

        def finish(g):
            gs, ge = gbounds[g]
            gsz = ge - gs
            both = singles.tile([P, 2, gsz], f32, name=f"both{g}", tag=f"both{g}")
            nc.sync.dma_start(
                out=both, in_=cc_out[g][:].rearrange("g p k -> p g k")
            )
            tot = singles.tile([P, gsz], f32, name=f"tot{g}", tag=f"tot{g}")
            nc.vector.tensor_add(tot, both[:, 0, :], both[:, 1, :])
            nc.vector.reciprocal(invr[:, gs:ge], tot)
            for ki in range(gs, ge):
                nc.vector.tensor_scalar_mul(
                    e_all[:, ki, :], e_all[:, ki, :], invr[:, ki : ki + 1]
                )
            for ki in range(gs, ge):
                for mb in range(MBT):
                    nc.tensor.matmul(
                        ps_tiles[mb][:],
                        lhsT=xt_all[:, ki, mb * P : (mb + 1) * P],
                        rhs=e_all[:, ki, :],
                        start=(ki == 0),
                        stop=(ki == KT - 1),
                    )

        # Software-pipelined emission.  chunk kb's sub is emitted after chunk
        # kb+1's front, its Exp right after.  Exchanges fire as soon as a
        # group's sums are complete; finish(g) (normalize + GEMM) is emitted
        # ~1 chunk later so the vector queue never stalls long on the CC
        # round trip.
        done_k = 0
        next_g = 0
        pend_fin = []

        def maybe_exchange():
            nonlocal next_g
            while next_g < NG and gbounds[next_g][1] <= done_k:
                exchange(next_g)
                pend_fin.append(next_g)
                next_g += 1

        prev = None
        for kb in range(NCH):
            fr = chunk_front(kb)
            if prev is not None:
                pkb, pu, pw = prev
                chunk_mid(pkb, pu, pw)
                if pkb == 0:
                    xt_load(0)
                    xt_load(1)
                elif pkb >= 1:
                    xt_load(pkb + 1)
                chunk_exp(pkb, pu)
                done_k = (pkb + 1) * KG
                maybe_exchange()
                for g in list(pend_fin):
                    if FIN_AT.get(g) == pkb:
                        finish(g)
                        pend_fin.remove(g)
            prev = (kb, fr[0], fr[1])
        pkb, pu, pw = prev
        chunk_mid(pkb, pu, pw)
        chunk_exp(pkb, pu)
        done_k = KT
        maybe_exchange()
        for g in pend_fin:
            finish(g)

        # Drain PSUM (f32 -> bf16) and store.  All 8 banks finish at the
        # last ktile simultaneously, so split the drain across vector AND
        # scalar (both can read PSUM) and alternate store queues to halve
        # the serial tail.
        for mb in range(MBT):
            o_t = outp.tile([P, OH], bf16, tag=f"o{mb % 2}")
            if mb % 2 == 0:
                nc.vector.tensor_copy(o_t, ps_tiles[mb][:])
                nc.sync.dma_start(out=out_d[mb * P : (mb + 1) * P, :], in_=o_t)
            else:
                nc.scalar.activation(
                    o_t, ps_tiles[mb][:],
                    mybir.ActivationFunctionType.Copy,
                )
                nc.scalar.dma_start(
                    out=out_d[mb * P : (mb + 1) * P, :], in_=o_t
                )

    nc.compile()
    return nc


def kernel(x, weight, uniform, T):
    global _PROGRAM, LAST_RESULT
    import ml_dtypes
    from concourse.bass_utils import run_bass_kernel_spmd

    if _PROGRAM is None:
        _PROGRAM = _build_program()
    nc = _PROGRAM

    bf = ml_dtypes.bfloat16
    x = np.asarray(x, dtype=np.float32)
    weight = np.asarray(weight, dtype=np.float32)
    uniform = np.asarray(uniform, dtype=np.float32)
    T = np.ascontiguousarray(np.asarray(T, dtype=np.float32)).reshape([1])

    xt = np.ascontiguousarray(x.T.astype(bf))  # [IN, B] bf16
    wb = weight.astype(bf)
    # v' = (1-u) * 2^14 in fp16: full relative precision at the u->1 tail
    # (which dominates the softmax) without any fp16 subnormals.
    vq = ((1.0 - uniform.astype(np.float64)) * VSCALE).astype(np.float16)
    vq = np.maximum(vq, np.float16(2.0**-10))
    in_maps = []
    for c in range(NCORES):
        p, q = c // GO, c % GO
        in_maps.append(
            {
                "xt": np.ascontiguousarray(xt[:, p * BS : (p + 1) * BS]),
                "wh": np.ascontiguousarray(wb[:, q * OH : (q + 1) * OH]),
                "uh": np.ascontiguousarray(vq[:, q * OH : (q + 1) * OH]),
                "tt": T,
            }
        )

    res = run_bass_kernel_spmd(nc, in_maps, core_ids=list(range(NCORES)))
    LAST_RESULT = res

    out = np.empty((B, OUT), dtype=np.float32)
    for c in range(NCORES):
        p, q = c // GO, c % GO
        out[p * BS : (p + 1) * BS, q * OH : (q + 1) * OH] = res.results[c][
            "out"
        ].astype(np.float32)
    return out
